# revision 19
# baseline (speedup 1.0000x reference)
"""Trainium2 Bass kernel for nn_DEA_GNN_JK (TAGConv x3 + JK-max + edge MLP scoring).

Strategy (8 NeuronCores, dst-sharded):
- Host relabels nodes: nodes are dealt to (core, slot) sorted by per-half padded
  chunk counts so the segment-sum slot structure is identical on every core.
- SpMM (A_norm @ h) per hop: dma_gather of bf16 rows from a per-core DRAM
  replica + PE matmul with a small constant one-hot stationary accumulating in
  PSUM. Row scalings (gcn_norm) are folded into per-node scales.
- The full h replica is refreshed per hop via AllGather of bf16 contributions.
- Dense TAGConv matmuls run node-major with transposed z-slabs (loaded via
  dma_gather(transpose=True)) as the PE stationary operand.
- JK max on DVE; candidate-edge scoring via feat-major MLP matmuls.

Host->device traffic is minimized (the axon tunnel moves ~30-60 MB/s): the
full-graph replica is NOT shipped (it is AllGathered on device from the
per-core contrib slabs), gather indices are shipped unreplicated ([16, W]
instead of the 8x-replicated [128, W] the gpsimd needs; replication happens
on-device with 8 partition-offset DMAs), and the replicated weight/constant
tensors are shipped as 1/8 shards that are AllGathered on device.
"""
import os
import sys
import time

sys.path.insert(0, "/opt/trn_rl_repo")

import numpy as np
import ml_dtypes

import concourse.bacc as bacc
import concourse.bass as bass
import concourse.mybir as mybir
import concourse.tile as tile
import concourse.tile_utils as tile_utils
from concourse.bass_utils import run_bass_kernel_spmd

BF16 = ml_dtypes.bfloat16

NCORES = 8
N = 50000
E2 = 65536
D = 256
KHOPS = 3
NLAYERS = 3
PER = 6250           # real nodes per core
PAD = 6272           # rows per core slab (49 * 128)
HALFROWS = 4 * PAD   # 25088
FULLROWS = 8 * PAD   # 50176
NTILES = PAD // 128  # 49
NGROUPS = PAD // 32  # 196
S = 4                # slots per dst per chunk (lane width)
ZIDX = PER           # zero row index within each half view (core0/core4 pad row)
MAXCH = 48           # max chunks per dma_gather call (48*128 = 6144 rows)
TBLK = 4             # tiles per gather block (chunk layout is block-half-major)
CAND_PER_CORE = E2 // NCORES

# --- column layout of the AllGathered constant blob [128, BLOB_COLS] bf16
WD_COLS = NLAYERS * (KHOPS + 1) * 2 * D   # 6144
W0P_OFF = WD_COLS                          # +512
W1X_OFF = W0P_OFF + 512                    # +64
MALL_OFF = W1X_OFF + 64                    # +512
BLOB_COLS = MALL_OFF + 512                 # 7232

OWN_COLS = PAD // 16                       # 392


def _pack_idx(idx):
    """[S] int16 -> [16, S//16]: slot i at (i%16, i//16). The gpsimd needs
    this replicated across the 8 Q7 16-partition blocks; replication happens
    on-device (8 partition-offset DMAs), not on the wire."""
    s = idx.shape[0]
    assert s % 16 == 0
    return np.ascontiguousarray(idx.reshape(s // 16, 16).T.astype(np.int16))


def _ranks_within_groups(key):
    """For each element, its occurrence index within its key group."""
    n = key.shape[0]
    order = np.argsort(key, kind="stable")
    sk = key[order]
    new_run = np.r_[True, sk[1:] != sk[:-1]]
    starts = np.flatnonzero(new_run)
    run_id = np.cumsum(new_run) - 1
    k_sorted = np.arange(n) - starts[run_id]
    k = np.empty(n, np.int64)
    k[order] = k_sorted
    return k


def _preprocess(x_feature, emb_weight, edge_index, edge_label_index):
    src = np.asarray(edge_index[0], dtype=np.int64)
    dst = np.asarray(edge_index[1], dtype=np.int64)

    deg = np.bincount(dst, minlength=N)
    deg_f = deg.astype(np.float32)
    dis = np.where(deg > 0, np.maximum(deg_f, np.float32(1.0)) ** np.float32(-0.5),
                   np.float32(0.0)).astype(np.float32)
    zscale = np.where(deg > 0, dis, np.float32(1.0)).astype(np.float32)

    # --- half assignment: alternate by degree rank -> 25000 per half
    order0 = np.argsort(-deg, kind="stable")
    half = np.zeros(N, np.int64)
    half[order0[1::2]] = 1

    # edges from isolated (deg==0) sources contribute weight 0 -> drop
    keep = deg[src] > 0
    srck, dstk = src[keep], dst[keep]
    h_e = half[srck]

    deg_lo = np.bincount(dstk[h_e == 0], minlength=N)
    deg_hi = np.bincount(dstk[h_e == 1], minlength=N)
    c_lo = -(-deg_lo // S)
    c_hi = -(-deg_hi // S)

    # --- deal nodes within each half to (core, slot), sorted so groups of 32
    # slots have homogeneous (c_lo, c_hi)
    core = np.zeros(N, np.int64)
    slot = np.zeros(N, np.int64)
    for h in (0, 1):
        nodes = np.flatnonzero(half == h)
        o = np.lexsort((-(deg_lo[nodes] + deg_hi[nodes]), -c_hi[nodes], -c_lo[nodes]))
        nodes = nodes[o]
        r = np.arange(nodes.shape[0])
        core[nodes] = 4 * h + (r % 4)
        slot[nodes] = r // 4
    row = core * PAD + slot

    # --- chunk counts per (group, half), shared across cores
    grp = slot // 32
    CH = np.zeros((NGROUPS, 2), np.int64)
    np.maximum.at(CH[:, 0], grp, c_lo)
    np.maximum.at(CH[:, 1], grp, c_hi)
    for t in range(NTILES):
        if CH[4 * t:4 * t + 4].sum() == 0:
            CH[4 * t, 0] = 1  # safety chunk so PSUM is always written

    # --- chunk layout: for block of TBLK tiles: for half: for tile: for
    # group: CH chunks. Gather calls batch whole (block, half) runs (up to
    # MAXCH chunks per call) to amortize the ~1us fixed SWDGE descriptor
    # generation cost on the Pool engine.
    CHUNK_START = np.zeros((NGROUPS, 2), np.int64)
    chunk_groups = []           # group id per global chunk
    tile_chunks = [[] for _ in range(NTILES)]   # (chunk_id, g) in MM order
    nblk = -(-NTILES // TBLK)
    calls = [[] for _ in range(nblk)]           # (h, c0, nch) gather calls
    blk_tiles = [range(b * TBLK, min(NTILES, (b + 1) * TBLK))
                 for b in range(nblk)]
    chunk2pos = {}              # chunk id -> (global call idx, row in call)
    ncalls = 0
    cidx = 0
    for b in range(nblk):
        for h in (0, 1):
            run0 = cidx
            for t in blk_tiles[b]:
                for g in range(4 * t, 4 * t + 4):
                    CHUNK_START[g, h] = cidx
                    for _ in range(int(CH[g, h])):
                        chunk_groups.append(g)
                        tile_chunks[t].append((cidx, g))
                        cidx += 1
            n, o = cidx - run0, run0
            while n > 0:
                take = min(n, MAXCH)
                calls[b].append((h, o, take))
                for r in range(take):
                    chunk2pos[o + r] = (ncalls, r)
                ncalls += 1
                o += take
                n -= take
    total_chunks = cidx
    s_total = total_chunks * 128

    # --- per-core slot index arrays
    k_e = _ranks_within_groups(dstk * 2 + h_e)
    g_e = grp[dstk]
    lane = (slot[dstk] % 32) * S + (k_e % S)
    pos = (CHUNK_START[g_e, h_e] + k_e // S) * 128 + lane
    val = (row[srck] - HALFROWS * h_e).astype(np.int16)
    assert (k_e // S < CH[g_e, h_e]).all()
    slots = np.full((NCORES, s_total), ZIDX, np.int16)
    slots.reshape(-1)[core[dstk] * s_total + pos] = val

    idx_seg = np.stack([_pack_idx(slots[c]) for c in range(NCORES)])

    # --- scales per (core, partition, tile)
    sc_zd = np.zeros((NCORES, 128, NTILES), np.float32)
    sc_inv = np.zeros((NCORES, 128, NTILES), np.float32)
    allnodes = np.arange(N)
    sc_zd[core, slot % 128, slot // 128] = (zscale * dis)[allnodes]
    sc_inv[core, slot % 128, slot // 128] = (np.float32(1.0) / zscale)[allnodes]

    # --- layer-1 z0 contributions (full replica is AllGathered on device)
    x0 = np.concatenate([np.asarray(emb_weight, np.float32),
                         np.asarray(x_feature, np.float32)], axis=1)
    z0 = x0 * zscale[:, None]
    slabs = np.zeros((NCORES, PAD, D), BF16)
    slabs[core, slot] = z0.astype(BF16)

    # --- candidate edges
    srcl = np.asarray(edge_label_index[0], dtype=np.int64)
    dstl = np.asarray(edge_label_index[1], dtype=np.int64)
    c_edge = np.arange(E2) // CAND_PER_CORE
    b_edge = 2 * half[srcl] + half[dstl]
    posc = _ranks_within_groups(c_edge * 4 + b_edge)
    bmax = int(posc.max()) + 1
    bcap = -(-bmax // 512) * 512
    candw = 4 * bcap

    cand = np.full((NCORES, 2, 4, bcap), ZIDX, np.int16)
    cand[c_edge, 0, b_edge, posc] = (row[srcl] - HALFROWS * half[srcl]).astype(np.int16)
    cand[c_edge, 1, b_edge, posc] = (row[dstl] - HALFROWS * half[dstl]).astype(np.int16)
    idx_cand = np.stack([_pack_idx(cand[c].reshape(-1)) for c in range(NCORES)])

    return dict(
        dis=dis, zscale=zscale, half=half, core=core, slot=slot, row=row,
        CH=CH, chunk_groups=chunk_groups, tile_chunks=tile_chunks, calls=calls,
        nblk=nblk, blk_tiles=blk_tiles, chunk2pos=chunk2pos,
        total_chunks=total_chunks, s_total=s_total,
        idx_seg=idx_seg, idx_cand=idx_cand, sc_zd=sc_zd, sc_inv=sc_inv,
        slabs=slabs,
        bcap=bcap, candw=candw, c_edge=c_edge, b_edge=b_edge, posc=posc,
    )


def _build_program(pp, dbg=False):
    s_total = pp["s_total"]
    tile_chunks = pp["tile_chunks"]
    calls = pp["calls"]
    nblk = pp["nblk"]
    blk_tiles = pp["blk_tiles"]
    chunk2pos = pp["chunk2pos"]
    chunk_groups = pp["chunk_groups"]
    bcap = pp["bcap"]
    candw = pp["candw"]

    # column layout of the [128, WTOT] int16 index tile
    seg_off = OWN_COLS
    cand_off = seg_off + s_total // 16
    wtot = cand_off + (8 * bcap) // 16

    f32 = mybir.dt.float32
    bf16 = mybir.dt.bfloat16
    i16 = mybir.dt.int16

    tile_utils.max_sbuf_usage = 206 * 1024

    nc = bacc.Bacc("TRN2", target_bir_lowering=False, debug=False,
                   num_devices=NCORES)
    RG = [list(range(NCORES))]

    # ---- I/O (kept minimal: the axon host->device path is ~30-60 MB/s)
    contrib0_in = nc.dram_tensor("contrib0", [PAD, D], bf16, kind="ExternalInput")
    idx_in = nc.dram_tensor("idx", [16, wtot], i16, kind="ExternalInput")
    blob_in = nc.dram_tensor("blob", [16, BLOB_COLS], bf16, kind="ExternalInput")
    sc_in = nc.dram_tensor("sc", [128, 2 * NTILES], f32, kind="ExternalInput")

    scores_out = nc.dram_tensor("scores", [1, candw], f32, kind="ExternalOutput")
    dbg_out = None
    if dbg:
        dbg_out = nc.dram_tensor("dbg", [128, NTILES, D], f32, kind="ExternalOutput")

    relu = mybir.ActivationFunctionType.Relu
    copyf = mybir.ActivationFunctionType.Copy

    with tile.TileContext(nc) as tc:
        with (
            tc.tile_pool(name="const", bufs=1) as cp,
            tc.tile_pool(name="dram", bufs=1, space="DRAM") as dp,
            tc.tile_pool(name="ps", bufs=4, space="PSUM") as psp,
            tc.tile_pool(name="slab", bufs=2) as slp,
        ):
            # ---- reconstruct replicated constants on device
            # (collectives cannot read IO tensors: stage via internal DRAM)
            blob_i = dp.tile([16, BLOB_COLS], bf16, tag="blob_i")
            nc.sync.dma_start(blob_i[:, :], blob_in.ap())
            c0i = dp.tile([PAD, D], bf16, tag="c0i")
            nc.sync.dma_start(c0i[:, :], contrib0_in.ap())
            blob_d = dp.tile([128, BLOB_COLS], bf16, addr_space="Shared",
                             tag="blob_d")
            nc.gpsimd.collective_compute(
                "AllGather", mybir.AluOpType.bypass, replica_groups=RG,
                ins=[blob_i.opt()], outs=[blob_d.opt()])
            # dma_gather from Shared-scratchpad DRAM runs ~10x slower than
            # from local DRAM (measured 76 vs >750 GB/s), so every AllGather
            # result is copied to a local DRAM tile before being gathered.
            def localize(shared_tile, tag):
                loc = dp.tile([FULLROWS, D], bf16, tag=tag)
                nc.sync.dma_start(loc[:, :], shared_tile[:])
                return loc

            r0g = dp.tile([FULLROWS, D], bf16, addr_space="Shared", tag="r0g")
            nc.gpsimd.collective_compute(
                "AllGather", mybir.AluOpType.bypass, replica_groups=RG,
                ins=[c0i.opt()], outs=[r0g.opt()])
            r0gl = localize(r0g, "r0gl")

            blob_sb = cp.tile([128, BLOB_COLS], bf16)
            nc.sync.dma_start(blob_sb[:], blob_d[:])
            idx_sb = cp.tile([128, wtot], i16)
            for kb in range(8):
                nc.sync.dma_start(idx_sb[16 * kb:16 * (kb + 1), :], idx_in[:])
            sc_sb = cp.tile([128, 2 * NTILES], f32)
            nc.sync.dma_start(sc_sb[:], sc_in[:])

            # typed views into the constant blob
            def wd_v(layer, k, fib):
                o = ((layer * (KHOPS + 1) + k) * 2 + fib) * D
                return blob_sb[:, o:o + D]

            def w0p_v(fib, fob):
                o = W0P_OFF + (fib * 2 + fob) * 128
                return blob_sb[:, o:o + 128]

            def w1x_v(fob):
                o = W1X_OFF + fob * 32
                return blob_sb[:, o:o + 32]

            def m_all_v(g):
                o = MALL_OFF + (g % 4) * 128
                return blob_sb[:, o:o + 128]

            lo_of = lambda rep: rep[0:HALFROWS, :]
            hi_of = lambda rep: rep[HALFROWS:FULLROWS, :]

            def hop(rep_prev_lo, rep_prev_hi, contrib_tile, gp, psh, tag):
                """One SpMM hop: gather + one-hot matmuls + scale -> z slab ->
                contrib DRAM. Returns nothing (contrib_tile written)."""
                zsl = slp.tile([128, NTILES, D], bf16, tag="zslab")
                gts = {}
                for b in range(nblk):
                    for (h, c0, nch) in calls[b]:
                        gt = gp.tile([128, MAXCH, D], bf16, tag=tag, bufs=3)
                        src_ap = rep_prev_lo if h == 0 else rep_prev_hi
                        nc.gpsimd.dma_gather(
                            gt[:, 0:nch, :], src_ap,
                            idx_sb[:, seg_off + c0 * 8:seg_off + (c0 + nch) * 8],
                            nch * 128, nch * 128, D, single_packet=False)
                        gts[chunk2pos[c0][0]] = gt
                    for t in blk_tiles[b]:
                        ps = psh.tile([128, D], f32, tag="acc", bufs=4)
                        ntot = len(tile_chunks[t])
                        for i, (c, g) in enumerate(tile_chunks[t]):
                            ci, r = chunk2pos[c]
                            nc.tensor.matmul(
                                ps[:], m_all_v(g), gts[ci][:, r, :],
                                start=(i == 0), stop=(i == ntot - 1))
                        nc.scalar.activation(zsl[:, t, :], ps[:], copyf,
                                             scale=sc_sb[:, t:t + 1])
                nc.sync.dma_start(
                    contrib_tile[:, :].rearrange("(t p) f -> p t f", p=128),
                    zsl[:, :, :])
                return zsl

            def dense_layer(layer, contrib_aps, ztp, z0n_out):
                """acc = sum_k z_k @ W[l,k]; z0n_out[:, nb, :] = relu(acc)."""
                zts = []
                for k in range(KHOPS + 1):
                    zt = ztp.tile([128, 2, PAD], bf16, tag=f"zT{k}")
                    nc.gpsimd.dma_gather(zt[:], contrib_aps[k],
                                         idx_sb[:, 0:OWN_COLS],
                                         PAD, PAD, D, transpose=True, single_packet=False)
                    zts.append(zt)
                for nb in range(NTILES):
                    ps = psp.tile([128, D], f32, tag="acc")
                    i = 0
                    for k in range(KHOPS + 1):
                        for fib in range(2):
                            nc.tensor.matmul(
                                ps[:],
                                zts[k][:, fib, nb * 128:(nb + 1) * 128],
                                wd_v(layer, k, fib),
                                start=(i == 0), stop=(i == 7))
                            i += 1
                    nc.scalar.activation(z0n_out[:, nb, :], ps[:], relu)

            jkz = cp.tile([128, NTILES, D], bf16, tag="jkz")

            contrib0_ap = contrib0_in.ap()
            rep_prev_lo, rep_prev_hi = lo_of(r0gl[:]), hi_of(r0gl[:])
            for layer in range(NLAYERS):
                contrib_aps = [contrib0_ap]
                with (
                    tc.tile_pool(name=f"g{layer}", bufs=3) as gp,
                    tc.tile_pool(name=f"ph{layer}", bufs=4, space="PSUM") as psh,
                ):
                    for k in range(1, KHOPS + 1):
                        ct = dp.tile([PAD, D], bf16, tag=f"c_l{layer}k{k}")
                        hop(rep_prev_lo, rep_prev_hi, ct, gp, psh, tag="G")
                        contrib_aps.append(ct[:, :])
                        if k < KHOPS:
                            if os.environ.get("KERNEL_ABLATE") == "noag":
                                continue
                            rp = dp.tile([FULLROWS, D], bf16,
                                         addr_space="Shared", tag=f"r_l{layer}k{k}")
                            nc.gpsimd.collective_compute(
                                "AllGather", mybir.AluOpType.bypass,
                                replica_groups=RG,
                                ins=[ct.opt()], outs=[rp.opt()])
                            rpl = localize(rp, f"rl_l{layer}k{k}")
                            rep_prev_lo, rep_prev_hi = lo_of(rpl[:]), hi_of(rpl[:])

                z0n = slp.tile([128, NTILES, D], bf16, tag="zslab")
                with tc.tile_pool(name=f"zt{layer}", bufs=1) as ztp:
                    dense_layer(layer, contrib_aps, ztp, z0n)

                if layer == 0:
                    nc.vector.tensor_copy(jkz[:], z0n[:])
                else:
                    nc.vector.tensor_tensor(jkz[:], jkz[:], z0n[:],
                                            op=mybir.AluOpType.max)

                if layer < NLAYERS - 1:
                    c0t = dp.tile([PAD, D], bf16, tag=f"c_l{layer + 1}k0")
                    nc.sync.dma_start(
                        c0t[:, :].rearrange("(t p) f -> p t f", p=128), z0n[:])
                    contrib0_ap = c0t[:, :]
                    if os.environ.get("KERNEL_ABLATE") == "noag":
                        continue
                    r0 = dp.tile([FULLROWS, D], bf16, addr_space="Shared",
                                 tag=f"r_l{layer + 1}k0")
                    nc.gpsimd.collective_compute(
                        "AllGather", mybir.AluOpType.bypass, replica_groups=RG,
                        ins=[c0t.opt()], outs=[r0.opt()])
                    r0l = localize(r0, f"rl_l{layer + 1}k0")
                    rep_prev_lo, rep_prev_hi = lo_of(r0l[:]), hi_of(r0l[:])

            # ---- JK output -> unscale -> AllGather
            jkc = slp.tile([128, NTILES, D], bf16, tag="zslab")
            for t in range(NTILES):
                nc.scalar.activation(jkc[:, t, :], jkz[:, t, :], copyf,
                                     scale=sc_sb[:, NTILES + t:NTILES + t + 1])
            cjk = dp.tile([PAD, D], bf16, tag="c_jk")
            nc.sync.dma_start(cjk[:, :].rearrange("(t p) f -> p t f", p=128), jkc[:])
            rjk_s = dp.tile([FULLROWS, D], bf16, addr_space="Shared", tag="r_jk")
            nc.gpsimd.collective_compute(
                "AllGather", mybir.AluOpType.bypass, replica_groups=RG,
                ins=[cjk.opt()], outs=[rjk_s.opt()])
            rjk = localize(rjk_s, "r_jkl")

            if dbg:
                jkf = slp.tile([128, NTILES, D], f32, tag="dbgf")
                nc.vector.tensor_copy(jkf[:], jkc[:])
                nc.sync.dma_start(dbg_out[:], jkf[:])

            # ---- candidate scoring
            with (
                tc.tile_pool(name="cand", bufs=1) as cnp,
                tc.tile_pool(name="psm", bufs=2, space="PSUM") as psm,
            ):
                scores_sb = cnp.tile([1, candw], f32)
                for b in range(4):
                    gsrc = cnp.tile([128, 2, bcap], bf16, tag=f"gs{b % 2}")
                    gdst = cnp.tile([128, 2, bcap], bf16, tag=f"gd{b % 2}")
                    h1t = cnp.tile([128, 2, bcap], bf16, tag=f"h1{b % 2}")
                    for side, gt in ((0, gsrc), (1, gdst)):
                        h = (b // 2) if side == 0 else (b % 2)
                        src_ap = lo_of(rjk[:]) if h == 0 else hi_of(rjk[:])
                        col0 = (side * 4 + b) * bcap
                        nc.gpsimd.dma_gather(
                            gt[:], src_ap,
                            idx_sb[:, cand_off + col0 // 16:
                                   cand_off + (col0 + bcap) // 16],
                            bcap, bcap, D, transpose=True, single_packet=False)
                    nc.vector.tensor_tensor(gsrc[:], gsrc[:], gdst[:],
                                            op=mybir.AluOpType.mult)
                    for c0 in range(0, bcap, 512):
                        for fob in range(2):
                            ph = psm.tile([128, 512], f32, tag="mlp")
                            for fib in range(2):
                                nc.tensor.matmul(
                                    ph[:], w0p_v(fib, fob),
                                    gsrc[:, fib, c0:c0 + 512],
                                    start=(fib == 0), stop=(fib == 1))
                            nc.scalar.activation(h1t[:, fob, c0:c0 + 512],
                                                 ph[:], relu)
                        pss = psm.tile([32, 512], f32, tag="sc")
                        for fob in range(2):
                            nc.tensor.matmul(
                                pss[:], w1x_v(fob),
                                h1t[:, fob, c0:c0 + 512],
                                start=(fob == 0), stop=(fob == 1))
                        nc.scalar.activation(
                            scores_sb[0:1, b * bcap + c0:b * bcap + c0 + 512],
                            pss[0:1, :], copyf)
                nc.sync.dma_start(scores_out[:], scores_sb[:])

    nc.compile()
    return nc


def _steady_state_exec_ns(nc, in_maps, n_iters=8):
    """Min wall-clock of repeated NEFF executions with inputs device-resident
    and the PJRT executable warm — the closest available stand-in for
    neuron-profile HW exec time in this axon build (no NTFF hook). Still an
    upper bound: it includes axon dispatch latency. Mirrors the lowering in
    concourse.bass2jax.run_bass_via_pjrt."""
    import jax
    from jax.sharding import Mesh, PartitionSpec, NamedSharding
    from jax.experimental.shard_map import shard_map
    from concourse.bass2jax import (
        _bass_exec_p, partition_id_tensor, install_neuronx_cc_hook)

    install_neuronx_cc_hook()
    assert nc.dbg_addr is None
    n_cores = len(in_maps)
    partition_name = nc.partition_id_tensor.name if nc.partition_id_tensor else None
    in_names, out_names, out_avals, zero_outs = [], [], [], []
    for alloc in nc.m.functions[0].allocations:
        if not isinstance(alloc, mybir.MemoryLocationSet):
            continue
        name = alloc.memorylocations[0].name
        if alloc.kind == "ExternalInput":
            if name != partition_name:
                in_names.append(name)
        elif alloc.kind == "ExternalOutput":
            out_names.append(name)
            shape = tuple(alloc.tensor_shape)
            dtype = mybir.dt.np(alloc.dtype)
            out_avals.append(jax.core.ShapedArray(shape, dtype))
            zero_outs.append(np.zeros(shape, dtype))
    n_params = len(in_names)
    n_outs = len(out_avals)
    in_names.extend(out_names)
    if partition_name is not None:
        in_names.append(partition_name)
    donate = tuple(range(n_params, n_params + n_outs))

    def _body(*args):
        operands = list(args)
        if partition_name is not None:
            operands.append(partition_id_tensor())
        outs = _bass_exec_p.bind(
            *operands, out_avals=tuple(out_avals), in_names=tuple(in_names),
            out_names=tuple(out_names), lowering_input_output_aliases=(),
            sim_require_finite=True, sim_require_nnan=True, nc=nc)
        return tuple(outs)

    devices = jax.devices()[:n_cores]
    mesh = Mesh(np.asarray(devices), ("core",))
    in_specs = (PartitionSpec("core"),) * (n_params + n_outs)
    out_specs = (PartitionSpec("core"),) * len(out_names)
    sharded = jax.jit(
        shard_map(_body, mesh=mesh, in_specs=in_specs, out_specs=out_specs,
                  check_rep=False),
        donate_argnums=donate, keep_unused=True)

    sh = NamedSharding(mesh, PartitionSpec("core"))
    concat_in = [
        jax.device_put(
            np.concatenate([np.asarray(m[name]) for m in in_maps], axis=0), sh)
        for name in in_names[:n_params]
    ]
    jax.block_until_ready(concat_in)

    def _zeros():
        zs = [
            jax.device_put(
                np.zeros((n_cores * z.shape[0], *z.shape[1:]), z.dtype), sh)
            for z in zero_outs
        ]
        jax.block_until_ready(zs)
        return zs

    out = sharded(*concat_in, *_zeros())  # warm-up (trace+compile)
    jax.block_until_ready(out)
    warm_out = [np.asarray(o).reshape(n_cores, *out_avals[i].shape)
                for i, o in enumerate(out)]

    # Pipelined amortized timing: dispatch K executions back-to-back (PJRT
    # queues them; the device runs them serially), block once, divide. This
    # amortizes the ~75 ms axon dispatch round-trip that a single timed
    # execution would include, so per-exec time upper-bounds true device time
    # much more tightly.
    best = None
    for _ in range(3):
        zsets = [_zeros() for _ in range(n_iters)]
        t0 = time.perf_counter()
        outs = [sharded(*concat_in, *zs) for zs in zsets]
        jax.block_until_ready(outs)
        dt = (time.perf_counter() - t0) / n_iters
        best = dt if best is None else min(best, dt)
    return int(best * 1e9), dict(zip(out_names, warm_out))


def kernel(**inputs):
    x_feature = np.asarray(inputs["x_feature"], np.float32)
    emb_weight = np.asarray(inputs["emb_weight"], np.float32)
    Ws = [np.asarray(inputs[f"W{i}"], np.float32) for i in range(3)]
    bs = [np.asarray(inputs[f"b{i}"], np.float32) for i in range(3)]
    mlp_w0 = np.asarray(inputs["mlp_w0"], np.float32)
    mlp_b0 = np.asarray(inputs["mlp_b0"], np.float32)
    mlp_w1 = np.asarray(inputs["mlp_w1"], np.float32)
    mlp_b1 = np.asarray(inputs["mlp_b1"], np.float32)
    edge_index = np.asarray(inputs["edge_index"])
    edge_label_index = np.asarray(inputs["edge_label_index"])

    for b in bs:
        assert np.all(b == 0), "nonzero TAGConv bias not supported"
    assert np.all(mlp_b0 == 0), "nonzero mlp bias not supported"

    pp = _preprocess(x_feature, emb_weight, edge_index, edge_label_index)
    nc = _build_program(pp)

    # ---- pack weights + constants into the sharded blob [128, BLOB_COLS]
    W = np.stack(Ws)  # [3, 4, 256, 256]
    wd = np.ascontiguousarray(
        W.reshape(NLAYERS, KHOPS + 1, 2, 128, D).transpose(3, 0, 1, 2, 4)
    ).astype(BF16)
    w0p = np.ascontiguousarray(
        mlp_w0.reshape(2, 128, 2, 128).transpose(1, 0, 2, 3)).astype(BF16)
    w1x = np.zeros((128, 2, 32), BF16)
    w1x[:, 0, 0] = mlp_w1[0:128, 0].astype(BF16)
    w1x[:, 1, 0] = mlp_w1[128:256, 0].astype(BF16)
    m_all = np.zeros((128, 4, 128), BF16)
    sidx = np.arange(128)
    for g in range(4):
        m_all[sidx, g, 32 * g + sidx // 4] = 1
    blob = np.concatenate([
        wd.reshape(128, WD_COLS), w0p.reshape(128, 512),
        w1x.reshape(128, 64), m_all.reshape(128, 512)], axis=1)
    assert blob.shape[1] == BLOB_COLS

    idx_own = _pack_idx(np.arange(PAD, dtype=np.int16))

    in_maps = []
    for c in range(NCORES):
        in_maps.append(dict(
            blob=np.ascontiguousarray(blob[16 * c:16 * (c + 1)]),
            contrib0=np.ascontiguousarray(pp["slabs"][c]),
            idx=np.ascontiguousarray(np.concatenate(
                [idx_own, pp["idx_seg"][c], pp["idx_cand"][c]], axis=1)),
            sc=np.ascontiguousarray(np.concatenate(
                [pp["sc_zd"][c], pp["sc_inv"][c]], axis=1)),
        ))

    t0 = time.time()
    res = run_bass_kernel_spmd(nc, in_maps, core_ids=list(range(NCORES)))
    e2e = time.time() - t0

    if os.environ.get("KERNEL_TRACE", "") == "1":
        exec_ns, warm_out = _steady_state_exec_ns(nc, in_maps, n_iters=48)
        ok = all(
            np.array_equal(warm_out["scores"][c], res.results[c]["scores"])
            for c in range(NCORES))
        print(f"HW exec time: {exec_ns} ns")
        print("  (per-execution time of the NEFF, measured as K pipelined "
              "back-to-back executions / K with inputs device-resident; no "
              "NTFF hook in this axon build — this upper-bounds true device "
              f"time; timing-path outputs match run_bass_kernel_spmd: {ok})")
        print(f"  end-to-end run_bass_kernel_spmd wall-clock incl. "
              f"host->device IO: {e2e:.2f} s")

    out = np.zeros(E2, np.float32)
    scores = np.stack([res.results[c]["scores"][0] for c in range(NCORES)])
    out = scores[pp["c_edge"], pp["b_edge"] * pp["bcap"] + pp["posc"]]
    out = out + np.float32(mlp_b1[0])
    return out.astype(np.float32)


if __name__ == "__main__":
    # smoke test with random data
    rng = np.random.default_rng(0)
    demo = {
        "x_feature": rng.standard_normal((N, 128), dtype=np.float32),
        "emb_weight": rng.standard_normal((N, 128), dtype=np.float32) * 0.05,
        "edge_index": rng.integers(0, N, (2, 800000)),
        "edge_label_index": rng.integers(0, N, (2, E2)),
        "mlp_w0": rng.standard_normal((D, D), dtype=np.float32) * 0.05,
        "mlp_b0": np.zeros(D, np.float32),
        "mlp_w1": rng.standard_normal((D, 1), dtype=np.float32) * 0.05,
        "mlp_b1": np.zeros(1, np.float32),
    }
    for i in range(3):
        demo[f"W{i}"] = rng.standard_normal((4, D, D), dtype=np.float32) * 0.05
        demo[f"b{i}"] = np.zeros((4, D), np.float32)
    out = kernel(**demo)
    print("out", out.shape, out[:8])


# revision 20
# speedup vs baseline: 1.0998x; 1.0998x over previous
"""Trainium2 Bass kernel for nn_DEA_GNN_JK (TAGConv x3 + JK-max + edge MLP scoring).

Strategy (8 NeuronCores, dst-sharded):
- Host relabels nodes: nodes are dealt to (core, slot) sorted by per-half padded
  chunk counts so the segment-sum slot structure is identical on every core.
- SpMM (A_norm @ h) per hop: dma_gather of bf16 rows from a per-core DRAM
  replica + PE matmul with a small constant one-hot stationary accumulating in
  PSUM. Row scalings (gcn_norm) are folded into per-node scales.
- The full h replica is refreshed per hop via AllGather of bf16 contributions.
- Dense TAGConv matmuls run node-major with transposed z-slabs (loaded via
  dma_gather(transpose=True)) as the PE stationary operand.
- JK max on DVE; candidate-edge scoring via feat-major MLP matmuls.

Host->device traffic is minimized (the axon tunnel moves ~30-60 MB/s): the
full-graph replica is NOT shipped (it is AllGathered on device from the
per-core contrib slabs), gather indices are shipped unreplicated ([16, W]
instead of the 8x-replicated [128, W] the gpsimd needs; replication happens
on-device with 8 partition-offset DMAs), and the replicated weight/constant
tensors are shipped as 1/8 shards that are AllGathered on device.
"""
import os
import sys
import time

sys.path.insert(0, "/opt/trn_rl_repo")

import numpy as np
import ml_dtypes

import concourse.bacc as bacc
import concourse.bass as bass
import concourse.mybir as mybir
import concourse.tile as tile
import concourse.tile_utils as tile_utils
from concourse.bass_utils import run_bass_kernel_spmd

BF16 = ml_dtypes.bfloat16

NCORES = 8
N = 50000
E2 = 65536
D = 256
KHOPS = 3
NLAYERS = 3
PER = 6250           # real nodes per core
PAD = 6272           # rows per core slab (49 * 128)
HALFROWS = 4 * PAD   # 25088
FULLROWS = 8 * PAD   # 50176
NTILES = PAD // 128  # 49
NGROUPS = PAD // 32  # 196
S = 4                # slots per dst per chunk (lane width)
ZIDX = PER           # zero row index within each half view (core0/core4 pad row)
MAXCH = 48           # max chunks per dma_gather call (48*128 = 6144 rows)
TBLK = 4             # tiles per gather block (chunk layout is block-half-major)
CAND_PER_CORE = E2 // NCORES

# --- column layout of the AllGathered constant blob [128, BLOB_COLS] bf16
WD_COLS = NLAYERS * (KHOPS + 1) * 2 * D   # 6144
W0P_OFF = WD_COLS                          # +512
W1X_OFF = W0P_OFF + 512                    # +64
MALL_OFF = W1X_OFF + 64                    # +512
BLOB_COLS = MALL_OFF + 512                 # 7232

OWN_COLS = PAD // 16                       # 392


def _pack_idx(idx):
    """[S] int16 -> [16, S//16]: slot i at (i%16, i//16). The gpsimd needs
    this replicated across the 8 Q7 16-partition blocks; replication happens
    on-device (8 partition-offset DMAs), not on the wire."""
    s = idx.shape[0]
    assert s % 16 == 0
    return np.ascontiguousarray(idx.reshape(s // 16, 16).T.astype(np.int16))


def _ranks_within_groups(key):
    """For each element, its occurrence index within its key group."""
    n = key.shape[0]
    order = np.argsort(key, kind="stable")
    sk = key[order]
    new_run = np.r_[True, sk[1:] != sk[:-1]]
    starts = np.flatnonzero(new_run)
    run_id = np.cumsum(new_run) - 1
    k_sorted = np.arange(n) - starts[run_id]
    k = np.empty(n, np.int64)
    k[order] = k_sorted
    return k


def _preprocess(x_feature, emb_weight, edge_index, edge_label_index):
    src = np.asarray(edge_index[0], dtype=np.int64)
    dst = np.asarray(edge_index[1], dtype=np.int64)

    deg = np.bincount(dst, minlength=N)
    deg_f = deg.astype(np.float32)
    dis = np.where(deg > 0, np.maximum(deg_f, np.float32(1.0)) ** np.float32(-0.5),
                   np.float32(0.0)).astype(np.float32)
    zscale = np.where(deg > 0, dis, np.float32(1.0)).astype(np.float32)

    # --- half assignment: alternate by degree rank -> 25000 per half
    order0 = np.argsort(-deg, kind="stable")
    half = np.zeros(N, np.int64)
    half[order0[1::2]] = 1

    # edges from isolated (deg==0) sources contribute weight 0 -> drop
    keep = deg[src] > 0
    srck, dstk = src[keep], dst[keep]
    h_e = half[srck]

    deg_lo = np.bincount(dstk[h_e == 0], minlength=N)
    deg_hi = np.bincount(dstk[h_e == 1], minlength=N)
    c_lo = -(-deg_lo // S)
    c_hi = -(-deg_hi // S)

    # --- deal nodes within each half to (core, slot), sorted so groups of 32
    # slots have homogeneous (c_lo, c_hi)
    core = np.zeros(N, np.int64)
    slot = np.zeros(N, np.int64)
    for h in (0, 1):
        nodes = np.flatnonzero(half == h)
        o = np.lexsort((-(deg_lo[nodes] + deg_hi[nodes]), -c_hi[nodes], -c_lo[nodes]))
        nodes = nodes[o]
        r = np.arange(nodes.shape[0])
        core[nodes] = 4 * h + (r % 4)
        slot[nodes] = r // 4
    row = core * PAD + slot

    # --- chunk counts per (group, half), shared across cores
    grp = slot // 32
    CH = np.zeros((NGROUPS, 2), np.int64)
    np.maximum.at(CH[:, 0], grp, c_lo)
    np.maximum.at(CH[:, 1], grp, c_hi)
    for t in range(NTILES):
        if CH[4 * t:4 * t + 4].sum() == 0:
            CH[4 * t, 0] = 1  # safety chunk so PSUM is always written

    # --- chunk layout: for block of TBLK tiles: for half: for tile: for
    # group: CH chunks. Gather calls batch whole (block, half) runs (up to
    # MAXCH chunks per call) to amortize the ~1us fixed SWDGE descriptor
    # generation cost on the Pool engine.
    CHUNK_START = np.zeros((NGROUPS, 2), np.int64)
    chunk_groups = []           # group id per global chunk
    tile_chunks = [[] for _ in range(NTILES)]   # (chunk_id, g) in MM order
    nblk = -(-NTILES // TBLK)
    calls = [[] for _ in range(nblk)]           # (h, c0, nch) gather calls
    blk_tiles = [range(b * TBLK, min(NTILES, (b + 1) * TBLK))
                 for b in range(nblk)]
    chunk2pos = {}              # chunk id -> (global call idx, row in call)
    ncalls = 0
    cidx = 0
    for b in range(nblk):
        for h in (0, 1):
            run0 = cidx
            for t in blk_tiles[b]:
                for g in range(4 * t, 4 * t + 4):
                    CHUNK_START[g, h] = cidx
                    for _ in range(int(CH[g, h])):
                        chunk_groups.append(g)
                        tile_chunks[t].append((cidx, g))
                        cidx += 1
            n, o = cidx - run0, run0
            while n > 0:
                take = min(n, MAXCH)
                calls[b].append((h, o, take))
                for r in range(take):
                    chunk2pos[o + r] = (ncalls, r)
                ncalls += 1
                o += take
                n -= take
    total_chunks = cidx
    s_total = total_chunks * 128

    # --- per-core slot index arrays
    k_e = _ranks_within_groups(dstk * 2 + h_e)
    g_e = grp[dstk]
    lane = (slot[dstk] % 32) * S + (k_e % S)
    pos = (CHUNK_START[g_e, h_e] + k_e // S) * 128 + lane
    val = (row[srck] - HALFROWS * h_e).astype(np.int16)
    assert (k_e // S < CH[g_e, h_e]).all()
    slots = np.full((NCORES, s_total), ZIDX, np.int16)
    slots.reshape(-1)[core[dstk] * s_total + pos] = val

    idx_seg = np.stack([_pack_idx(slots[c]) for c in range(NCORES)])

    # --- scales per (core, partition, tile)
    sc_zd = np.zeros((NCORES, 128, NTILES), np.float32)
    sc_inv = np.zeros((NCORES, 128, NTILES), np.float32)
    allnodes = np.arange(N)
    sc_zd[core, slot % 128, slot // 128] = (zscale * dis)[allnodes]
    sc_inv[core, slot % 128, slot // 128] = (np.float32(1.0) / zscale)[allnodes]

    # --- layer-1 z0 contributions (full replica is AllGathered on device)
    x0 = np.concatenate([np.asarray(emb_weight, np.float32),
                         np.asarray(x_feature, np.float32)], axis=1)
    z0 = x0 * zscale[:, None]
    slabs = np.zeros((NCORES, PAD, D), BF16)
    slabs[core, slot] = z0.astype(BF16)

    # --- candidate edges
    srcl = np.asarray(edge_label_index[0], dtype=np.int64)
    dstl = np.asarray(edge_label_index[1], dtype=np.int64)
    c_edge = np.arange(E2) // CAND_PER_CORE
    b_edge = 2 * half[srcl] + half[dstl]
    posc = _ranks_within_groups(c_edge * 4 + b_edge)
    bmax = int(posc.max()) + 1
    bcap = -(-bmax // 512) * 512
    candw = 4 * bcap

    cand = np.full((NCORES, 2, 4, bcap), ZIDX, np.int16)
    cand[c_edge, 0, b_edge, posc] = (row[srcl] - HALFROWS * half[srcl]).astype(np.int16)
    cand[c_edge, 1, b_edge, posc] = (row[dstl] - HALFROWS * half[dstl]).astype(np.int16)
    idx_cand = np.stack([_pack_idx(cand[c].reshape(-1)) for c in range(NCORES)])

    return dict(
        dis=dis, zscale=zscale, half=half, core=core, slot=slot, row=row,
        CH=CH, chunk_groups=chunk_groups, tile_chunks=tile_chunks, calls=calls,
        nblk=nblk, blk_tiles=blk_tiles, chunk2pos=chunk2pos,
        total_chunks=total_chunks, s_total=s_total,
        idx_seg=idx_seg, idx_cand=idx_cand, sc_zd=sc_zd, sc_inv=sc_inv,
        slabs=slabs,
        bcap=bcap, candw=candw, c_edge=c_edge, b_edge=b_edge, posc=posc,
    )


def _build_program(pp, dbg=False):
    s_total = pp["s_total"]
    tile_chunks = pp["tile_chunks"]
    calls = pp["calls"]
    nblk = pp["nblk"]
    blk_tiles = pp["blk_tiles"]
    chunk2pos = pp["chunk2pos"]
    chunk_groups = pp["chunk_groups"]
    bcap = pp["bcap"]
    candw = pp["candw"]

    # column layout of the [128, WTOT] int16 index tile
    seg_off = OWN_COLS
    cand_off = seg_off + s_total // 16
    wtot = cand_off + (8 * bcap) // 16

    f32 = mybir.dt.float32
    bf16 = mybir.dt.bfloat16
    i16 = mybir.dt.int16

    tile_utils.max_sbuf_usage = 206 * 1024

    nc = bacc.Bacc("TRN2", target_bir_lowering=False, debug=False,
                   num_devices=NCORES)
    RG = [list(range(NCORES))]

    # ---- I/O (kept minimal: the axon host->device path is ~30-60 MB/s)
    contrib0_in = nc.dram_tensor("contrib0", [PAD, D], bf16, kind="ExternalInput")
    idx_in = nc.dram_tensor("idx", [16, wtot], i16, kind="ExternalInput")
    blob_in = nc.dram_tensor("blob", [16, BLOB_COLS], bf16, kind="ExternalInput")
    sc_in = nc.dram_tensor("sc", [128, 2 * NTILES], f32, kind="ExternalInput")

    scores_out = nc.dram_tensor("scores", [1, candw], f32, kind="ExternalOutput")
    dbg_out = None
    if dbg:
        dbg_out = nc.dram_tensor("dbg", [128, NTILES, D], f32, kind="ExternalOutput")

    relu = mybir.ActivationFunctionType.Relu
    copyf = mybir.ActivationFunctionType.Copy

    with tile.TileContext(nc) as tc:
        with (
            tc.tile_pool(name="const", bufs=1) as cp,
            tc.tile_pool(name="dram", bufs=1, space="DRAM") as dp,
            tc.tile_pool(name="ps", bufs=4, space="PSUM") as psp,
            tc.tile_pool(name="slab", bufs=2) as slp,
        ):
            # ---- reconstruct replicated constants on device
            # (collectives cannot read IO tensors: stage via internal DRAM)
            blob_i = dp.tile([16, BLOB_COLS], bf16, tag="blob_i")
            nc.sync.dma_start(blob_i[:, :], blob_in.ap())
            c0i = dp.tile([PAD, D], bf16, tag="c0i")
            nc.sync.dma_start(c0i[:, :], contrib0_in.ap())
            blob_d = dp.tile([128, BLOB_COLS], bf16, addr_space="Shared",
                             tag="blob_d")
            nc.gpsimd.collective_compute(
                "AllGather", mybir.AluOpType.bypass, replica_groups=RG,
                ins=[blob_i.opt()], outs=[blob_d.opt()])
            r0g = dp.tile([FULLROWS, D], bf16, addr_space="Shared", tag="r0g")
            nc.gpsimd.collective_compute(
                "AllGather", mybir.AluOpType.bypass, replica_groups=RG,
                ins=[c0i.opt()], outs=[r0g.opt()])

            blob_sb = cp.tile([128, BLOB_COLS], bf16)
            nc.sync.dma_start(blob_sb[:], blob_d[:])
            idx_sb = cp.tile([128, wtot], i16)
            for kb in range(8):
                nc.sync.dma_start(idx_sb[16 * kb:16 * (kb + 1), :], idx_in[:])
            sc_sb = cp.tile([128, 2 * NTILES], f32)
            nc.sync.dma_start(sc_sb[:], sc_in[:])

            # typed views into the constant blob
            def wd_v(layer, k, fib):
                o = ((layer * (KHOPS + 1) + k) * 2 + fib) * D
                return blob_sb[:, o:o + D]

            def w0p_v(fib, fob):
                o = W0P_OFF + (fib * 2 + fob) * 128
                return blob_sb[:, o:o + 128]

            def w1x_v(fob):
                o = W1X_OFF + fob * 32
                return blob_sb[:, o:o + 32]

            def m_all_v(g):
                o = MALL_OFF + (g % 4) * 128
                return blob_sb[:, o:o + 128]

            lo_of = lambda rep: rep[0:HALFROWS, :]
            hi_of = lambda rep: rep[HALFROWS:FULLROWS, :]

            def hop(rep_prev_lo, rep_prev_hi, contrib_tile, gp, psh, tag):
                """One SpMM hop: gather + one-hot matmuls + scale -> z slab ->
                contrib DRAM. Returns nothing (contrib_tile written)."""
                zsl = slp.tile([128, NTILES, D], bf16, tag="zslab")
                gts = {}
                for b in range(nblk):
                    for (h, c0, nch) in calls[b]:
                        gt = gp.tile([128, MAXCH, D], bf16, tag=tag, bufs=3)
                        src_ap = rep_prev_lo if h == 0 else rep_prev_hi
                        nc.gpsimd.dma_gather(
                            gt[:, 0:nch, :], src_ap,
                            idx_sb[:, seg_off + c0 * 8:seg_off + (c0 + nch) * 8],
                            nch * 128, nch * 128, D, single_packet=False)
                        gts[chunk2pos[c0][0]] = gt
                    for t in blk_tiles[b]:
                        ps = psh.tile([128, D], f32, tag="acc", bufs=4)
                        ntot = len(tile_chunks[t])
                        for i, (c, g) in enumerate(tile_chunks[t]):
                            ci, r = chunk2pos[c]
                            nc.tensor.matmul(
                                ps[:], m_all_v(g), gts[ci][:, r, :],
                                start=(i == 0), stop=(i == ntot - 1))
                        nc.scalar.activation(zsl[:, t, :], ps[:], copyf,
                                             scale=sc_sb[:, t:t + 1])
                nc.sync.dma_start(
                    contrib_tile[:, :].rearrange("(t p) f -> p t f", p=128),
                    zsl[:, :, :])
                return zsl

            def dense_layer(layer, contrib_aps, ztp, z0n_out):
                """acc = sum_k z_k @ W[l,k]; z0n_out[:, nb, :] = relu(acc)."""
                zts = []
                for k in range(KHOPS + 1):
                    zt = ztp.tile([128, 2, PAD], bf16, tag=f"zT{k}")
                    nc.gpsimd.dma_gather(zt[:], contrib_aps[k],
                                         idx_sb[:, 0:OWN_COLS],
                                         PAD, PAD, D, transpose=True, single_packet=False)
                    zts.append(zt)
                for nb in range(NTILES):
                    ps = psp.tile([128, D], f32, tag="acc")
                    i = 0
                    for k in range(KHOPS + 1):
                        for fib in range(2):
                            nc.tensor.matmul(
                                ps[:],
                                zts[k][:, fib, nb * 128:(nb + 1) * 128],
                                wd_v(layer, k, fib),
                                start=(i == 0), stop=(i == 7))
                            i += 1
                    nc.scalar.activation(z0n_out[:, nb, :], ps[:], relu)

            jkz = cp.tile([128, NTILES, D], bf16, tag="jkz")

            contrib0_ap = contrib0_in.ap()
            rep_prev_lo, rep_prev_hi = lo_of(r0g[:]), hi_of(r0g[:])
            for layer in range(NLAYERS):
                contrib_aps = [contrib0_ap]
                with (
                    tc.tile_pool(name=f"g{layer}", bufs=3) as gp,
                    tc.tile_pool(name=f"ph{layer}", bufs=4, space="PSUM") as psh,
                ):
                    for k in range(1, KHOPS + 1):
                        ct = dp.tile([PAD, D], bf16, tag=f"c_l{layer}k{k}")
                        hop(rep_prev_lo, rep_prev_hi, ct, gp, psh, tag="G")
                        contrib_aps.append(ct[:, :])
                        if k < KHOPS:
                            if os.environ.get("KERNEL_ABLATE") == "noag":
                                continue
                            rp = dp.tile([FULLROWS, D], bf16,
                                         addr_space="Shared", tag=f"r_l{layer}k{k}")
                            nc.gpsimd.collective_compute(
                                "AllGather", mybir.AluOpType.bypass,
                                replica_groups=RG,
                                ins=[ct.opt()], outs=[rp.opt()])
                            rep_prev_lo, rep_prev_hi = lo_of(rp[:]), hi_of(rp[:])

                z0n = slp.tile([128, NTILES, D], bf16, tag="zslab")
                with tc.tile_pool(name=f"zt{layer}", bufs=1) as ztp:
                    dense_layer(layer, contrib_aps, ztp, z0n)

                if layer == 0:
                    nc.vector.tensor_copy(jkz[:], z0n[:])
                else:
                    nc.vector.tensor_tensor(jkz[:], jkz[:], z0n[:],
                                            op=mybir.AluOpType.max)

                if layer < NLAYERS - 1:
                    c0t = dp.tile([PAD, D], bf16, tag=f"c_l{layer + 1}k0")
                    nc.sync.dma_start(
                        c0t[:, :].rearrange("(t p) f -> p t f", p=128), z0n[:])
                    contrib0_ap = c0t[:, :]
                    if os.environ.get("KERNEL_ABLATE") == "noag":
                        continue
                    r0 = dp.tile([FULLROWS, D], bf16, addr_space="Shared",
                                 tag=f"r_l{layer + 1}k0")
                    nc.gpsimd.collective_compute(
                        "AllGather", mybir.AluOpType.bypass, replica_groups=RG,
                        ins=[c0t.opt()], outs=[r0.opt()])
                    rep_prev_lo, rep_prev_hi = lo_of(r0[:]), hi_of(r0[:])

            # ---- JK output -> unscale -> AllGather
            jkc = slp.tile([128, NTILES, D], bf16, tag="zslab")
            for t in range(NTILES):
                nc.scalar.activation(jkc[:, t, :], jkz[:, t, :], copyf,
                                     scale=sc_sb[:, NTILES + t:NTILES + t + 1])
            cjk = dp.tile([PAD, D], bf16, tag="c_jk")
            nc.sync.dma_start(cjk[:, :].rearrange("(t p) f -> p t f", p=128), jkc[:])
            rjk = dp.tile([FULLROWS, D], bf16, addr_space="Shared", tag="r_jk")
            nc.gpsimd.collective_compute(
                "AllGather", mybir.AluOpType.bypass, replica_groups=RG,
                ins=[cjk.opt()], outs=[rjk.opt()])

            if dbg:
                jkf = slp.tile([128, NTILES, D], f32, tag="dbgf")
                nc.vector.tensor_copy(jkf[:], jkc[:])
                nc.sync.dma_start(dbg_out[:], jkf[:])

            # ---- candidate scoring
            with (
                tc.tile_pool(name="cand", bufs=1) as cnp,
                tc.tile_pool(name="psm", bufs=2, space="PSUM") as psm,
            ):
                scores_sb = cnp.tile([1, candw], f32)
                for b in range(4):
                    gsrc = cnp.tile([128, 2, bcap], bf16, tag=f"gs{b % 2}")
                    gdst = cnp.tile([128, 2, bcap], bf16, tag=f"gd{b % 2}")
                    h1t = cnp.tile([128, 2, bcap], bf16, tag=f"h1{b % 2}")
                    for side, gt in ((0, gsrc), (1, gdst)):
                        h = (b // 2) if side == 0 else (b % 2)
                        src_ap = lo_of(rjk[:]) if h == 0 else hi_of(rjk[:])
                        col0 = (side * 4 + b) * bcap
                        nc.gpsimd.dma_gather(
                            gt[:], src_ap,
                            idx_sb[:, cand_off + col0 // 16:
                                   cand_off + (col0 + bcap) // 16],
                            bcap, bcap, D, transpose=True, single_packet=False)
                    nc.vector.tensor_tensor(gsrc[:], gsrc[:], gdst[:],
                                            op=mybir.AluOpType.mult)
                    for c0 in range(0, bcap, 512):
                        for fob in range(2):
                            ph = psm.tile([128, 512], f32, tag="mlp")
                            for fib in range(2):
                                nc.tensor.matmul(
                                    ph[:], w0p_v(fib, fob),
                                    gsrc[:, fib, c0:c0 + 512],
                                    start=(fib == 0), stop=(fib == 1))
                            nc.scalar.activation(h1t[:, fob, c0:c0 + 512],
                                                 ph[:], relu)
                        pss = psm.tile([32, 512], f32, tag="sc")
                        for fob in range(2):
                            nc.tensor.matmul(
                                pss[:], w1x_v(fob),
                                h1t[:, fob, c0:c0 + 512],
                                start=(fob == 0), stop=(fob == 1))
                        nc.scalar.activation(
                            scores_sb[0:1, b * bcap + c0:b * bcap + c0 + 512],
                            pss[0:1, :], copyf)
                nc.sync.dma_start(scores_out[:], scores_sb[:])

    nc.compile()
    return nc


def _steady_state_exec_ns(nc, in_maps, n_iters=8):
    """Min wall-clock of repeated NEFF executions with inputs device-resident
    and the PJRT executable warm — the closest available stand-in for
    neuron-profile HW exec time in this axon build (no NTFF hook). Still an
    upper bound: it includes axon dispatch latency. Mirrors the lowering in
    concourse.bass2jax.run_bass_via_pjrt."""
    import jax
    from jax.sharding import Mesh, PartitionSpec, NamedSharding
    from jax.experimental.shard_map import shard_map
    from concourse.bass2jax import (
        _bass_exec_p, partition_id_tensor, install_neuronx_cc_hook)

    install_neuronx_cc_hook()
    assert nc.dbg_addr is None
    n_cores = len(in_maps)
    partition_name = nc.partition_id_tensor.name if nc.partition_id_tensor else None
    in_names, out_names, out_avals, zero_outs = [], [], [], []
    for alloc in nc.m.functions[0].allocations:
        if not isinstance(alloc, mybir.MemoryLocationSet):
            continue
        name = alloc.memorylocations[0].name
        if alloc.kind == "ExternalInput":
            if name != partition_name:
                in_names.append(name)
        elif alloc.kind == "ExternalOutput":
            out_names.append(name)
            shape = tuple(alloc.tensor_shape)
            dtype = mybir.dt.np(alloc.dtype)
            out_avals.append(jax.core.ShapedArray(shape, dtype))
            zero_outs.append(np.zeros(shape, dtype))
    n_params = len(in_names)
    n_outs = len(out_avals)
    in_names.extend(out_names)
    if partition_name is not None:
        in_names.append(partition_name)
    donate = tuple(range(n_params, n_params + n_outs))

    def _body(*args):
        operands = list(args)
        if partition_name is not None:
            operands.append(partition_id_tensor())
        outs = _bass_exec_p.bind(
            *operands, out_avals=tuple(out_avals), in_names=tuple(in_names),
            out_names=tuple(out_names), lowering_input_output_aliases=(),
            sim_require_finite=True, sim_require_nnan=True, nc=nc)
        return tuple(outs)

    devices = jax.devices()[:n_cores]
    mesh = Mesh(np.asarray(devices), ("core",))
    in_specs = (PartitionSpec("core"),) * (n_params + n_outs)
    out_specs = (PartitionSpec("core"),) * len(out_names)
    sharded = jax.jit(
        shard_map(_body, mesh=mesh, in_specs=in_specs, out_specs=out_specs,
                  check_rep=False),
        donate_argnums=donate, keep_unused=True)

    sh = NamedSharding(mesh, PartitionSpec("core"))
    concat_in = [
        jax.device_put(
            np.concatenate([np.asarray(m[name]) for m in in_maps], axis=0), sh)
        for name in in_names[:n_params]
    ]
    jax.block_until_ready(concat_in)

    def _zeros():
        zs = [
            jax.device_put(
                np.zeros((n_cores * z.shape[0], *z.shape[1:]), z.dtype), sh)
            for z in zero_outs
        ]
        jax.block_until_ready(zs)
        return zs

    out = sharded(*concat_in, *_zeros())  # warm-up (trace+compile)
    jax.block_until_ready(out)
    warm_out = [np.asarray(o).reshape(n_cores, *out_avals[i].shape)
                for i, o in enumerate(out)]

    # Pipelined amortized timing: dispatch K executions back-to-back (PJRT
    # queues them; the device runs them serially), block once, divide. This
    # amortizes the ~75 ms axon dispatch round-trip that a single timed
    # execution would include, so per-exec time upper-bounds true device time
    # much more tightly.
    best = None
    for _ in range(3):
        zsets = [_zeros() for _ in range(n_iters)]
        t0 = time.perf_counter()
        outs = [sharded(*concat_in, *zs) for zs in zsets]
        jax.block_until_ready(outs)
        dt = (time.perf_counter() - t0) / n_iters
        best = dt if best is None else min(best, dt)
    return int(best * 1e9), dict(zip(out_names, warm_out))


def kernel(**inputs):
    x_feature = np.asarray(inputs["x_feature"], np.float32)
    emb_weight = np.asarray(inputs["emb_weight"], np.float32)
    Ws = [np.asarray(inputs[f"W{i}"], np.float32) for i in range(3)]
    bs = [np.asarray(inputs[f"b{i}"], np.float32) for i in range(3)]
    mlp_w0 = np.asarray(inputs["mlp_w0"], np.float32)
    mlp_b0 = np.asarray(inputs["mlp_b0"], np.float32)
    mlp_w1 = np.asarray(inputs["mlp_w1"], np.float32)
    mlp_b1 = np.asarray(inputs["mlp_b1"], np.float32)
    edge_index = np.asarray(inputs["edge_index"])
    edge_label_index = np.asarray(inputs["edge_label_index"])

    for b in bs:
        assert np.all(b == 0), "nonzero TAGConv bias not supported"
    assert np.all(mlp_b0 == 0), "nonzero mlp bias not supported"

    pp = _preprocess(x_feature, emb_weight, edge_index, edge_label_index)
    nc = _build_program(pp)

    # ---- pack weights + constants into the sharded blob [128, BLOB_COLS]
    W = np.stack(Ws)  # [3, 4, 256, 256]
    wd = np.ascontiguousarray(
        W.reshape(NLAYERS, KHOPS + 1, 2, 128, D).transpose(3, 0, 1, 2, 4)
    ).astype(BF16)
    w0p = np.ascontiguousarray(
        mlp_w0.reshape(2, 128, 2, 128).transpose(1, 0, 2, 3)).astype(BF16)
    w1x = np.zeros((128, 2, 32), BF16)
    w1x[:, 0, 0] = mlp_w1[0:128, 0].astype(BF16)
    w1x[:, 1, 0] = mlp_w1[128:256, 0].astype(BF16)
    m_all = np.zeros((128, 4, 128), BF16)
    sidx = np.arange(128)
    for g in range(4):
        m_all[sidx, g, 32 * g + sidx // 4] = 1
    blob = np.concatenate([
        wd.reshape(128, WD_COLS), w0p.reshape(128, 512),
        w1x.reshape(128, 64), m_all.reshape(128, 512)], axis=1)
    assert blob.shape[1] == BLOB_COLS

    idx_own = _pack_idx(np.arange(PAD, dtype=np.int16))

    in_maps = []
    for c in range(NCORES):
        in_maps.append(dict(
            blob=np.ascontiguousarray(blob[16 * c:16 * (c + 1)]),
            contrib0=np.ascontiguousarray(pp["slabs"][c]),
            idx=np.ascontiguousarray(np.concatenate(
                [idx_own, pp["idx_seg"][c], pp["idx_cand"][c]], axis=1)),
            sc=np.ascontiguousarray(np.concatenate(
                [pp["sc_zd"][c], pp["sc_inv"][c]], axis=1)),
        ))

    t0 = time.time()
    res = run_bass_kernel_spmd(nc, in_maps, core_ids=list(range(NCORES)))
    e2e = time.time() - t0

    if os.environ.get("KERNEL_TRACE", "") == "1":
        exec_ns, warm_out = _steady_state_exec_ns(nc, in_maps, n_iters=48)
        ok = all(
            np.array_equal(warm_out["scores"][c], res.results[c]["scores"])
            for c in range(NCORES))
        print(f"HW exec time: {exec_ns} ns")
        print("  (per-execution time of the NEFF, measured as K pipelined "
              "back-to-back executions / K with inputs device-resident; no "
              "NTFF hook in this axon build — this upper-bounds true device "
              f"time; timing-path outputs match run_bass_kernel_spmd: {ok})")
        print(f"  end-to-end run_bass_kernel_spmd wall-clock incl. "
              f"host->device IO: {e2e:.2f} s")

    out = np.zeros(E2, np.float32)
    scores = np.stack([res.results[c]["scores"][0] for c in range(NCORES)])
    out = scores[pp["c_edge"], pp["b_edge"] * pp["bcap"] + pp["posc"]]
    out = out + np.float32(mlp_b1[0])
    return out.astype(np.float32)


if __name__ == "__main__":
    # smoke test with random data
    rng = np.random.default_rng(0)
    demo = {
        "x_feature": rng.standard_normal((N, 128), dtype=np.float32),
        "emb_weight": rng.standard_normal((N, 128), dtype=np.float32) * 0.05,
        "edge_index": rng.integers(0, N, (2, 800000)),
        "edge_label_index": rng.integers(0, N, (2, E2)),
        "mlp_w0": rng.standard_normal((D, D), dtype=np.float32) * 0.05,
        "mlp_b0": np.zeros(D, np.float32),
        "mlp_w1": rng.standard_normal((D, 1), dtype=np.float32) * 0.05,
        "mlp_b1": np.zeros(1, np.float32),
    }
    for i in range(3):
        demo[f"W{i}"] = rng.standard_normal((4, D, D), dtype=np.float32) * 0.05
        demo[f"b{i}"] = np.zeros((4, D), np.float32)
    out = kernel(**demo)
    print("out", out.shape, out[:8])
